# revision 40
# baseline (speedup 1.0000x reference)
"""Two-branch attention kernel for Trainium2 (8 NeuronCores, batch-parallel).

out1 = proj(softmax(q k^T / 8) v),  out2 = proj(softmax(q k2^T / 8) v2)
with q,k,v from x and k2,v2 from x2 (q shared across branches).

Sharding: batch dim (8) -> one batch element per core. No collectives.

Design (per core, all matmul operands bf16, PSUM f32):
  QKV:  qT/kT [dim,tok] via W-stationary matmuls; v [tok,dim] via
        x-stationary matmuls into a ones-augmented buffer (AUG=65 column
        carries softmax row sums for free). k2/v2 kept in SBUF (no DRAM
        spill); their formation is interleaved into branch-1 attention so
        the PE fills the ACT-bound softmax gaps.
  Attn: per (kj,c): S^T chunk [128,2,512] in a double-buffered PSUM pair;
        the two head-half matmuls land on PE row-groups 0/64 and run
        concurrently. exp on ScalarE (scale=1/8, no max subtraction)
        writes P^T bf16; AV (V_aug stationary) pipelined one kj behind.
  Norm: po -> osb evict on DVE; row sums bounce DRAM -> [128,2,8] so the
        reciprocal uses all partitions (~0.1us, not 6.5us); broadcast back
        and multiply into ot (bf16).
  Proj: ot tiles stationary, stream W_proj^T, add bias on DVE, DMA out.
        proj(br0) interleaved into branch-2 attention.
"""
import sys
for _p in ('/opt/trn_rl_repo',):
    if _p not in sys.path:
        sys.path.insert(0, _p)

import numpy as np

MODE = 'bf16'

B, N, D, H, HD = 8, 1024, 768, 12, 64
SCALE = HD ** -0.5
NDT = D // 128       # 6 dim tiles
NQT = N // 128       # 8 token tiles
P = 128
AUG = HD + 1         # 65: head dim + ones column for row sums


# ----------------------------------------------------------------------------
# workaround: walrus rejects >2 sem waits on one instruction; TileContext's
# tail drain carries one wait per active logical proc. Split them across
# single-wait SP nops and emit a bare drain.
def _install_tilefix():
    import bass_rust
    import concourse.tile as tile

    def _drain_and_barrier_split(self, tick_clock, wait_clock):
        gc = tick_clock.global_clock
        ticks = [gc[i] for i in range(27)]
        for i, t in enumerate(ticks):
            if t > 0:
                vc = bass_rust.VectorClock(
                    [t if j == i else 0 for j in range(len(ticks))])
                nop = self.nc.sync.nop()
                wait_clock.add_sem_waits(
                    nop.ins, bass_rust.ScopedClock({None: vc}))
        self.nc.sync.drain()
        self.nc.all_engine_barrier()
        assert self.sems is not None
        popped = self.nc._tile_sem_poison_stack.pop()
        assert popped is self._sem_poison
        self.nc.clear_and_free_semaphores(list(self.sems.allocated().values()))
        self.nc.all_engine_barrier()

    tile.TileContext._drain_and_barrier = _drain_and_barrier_split


def _split_multiwaits(nc, max_waits=1):
    """walrus codegen rejects instructions carrying more than `max_waits`
    sync waits; hoist the extras onto same-engine nops placed just before."""
    import bass_rust
    import concourse.mybir as mybir
    cnt = 0
    for bb in nc.main_func.blocks:
        insts = bb.instructions
        i = 0
        while i < len(insts):
            ins = insts[i]
            si = getattr(ins, 'sync_info', None)
            if si is not None and si.on_wait and len(si.on_wait) > max_waits:
                waits = list(si.on_wait)
                extras, keep = waits[:-max_waits], waits[-max_waits:]
                for w in extras:
                    nop = mybir.InstNoOp(name=f"I-swx{cnt}", ins=[], outs=[])
                    cnt += 1
                    nop.engine = ins.engine
                    nop.sync_info = bass_rust.SyncInfo(on_wait=[w],
                                                       on_update=[])
                    insts.insert(i, nop)
                    i += 1
                ins.sync_info = bass_rust.SyncInfo(
                    on_wait=keep, on_update=list(si.on_update))
            i += 1
    return cnt


_built = None


def _build():
    """Build the SPMD bass program once. Returns (nc, n_split_waits)."""
    global _built
    if _built is not None:
        return _built
    _install_tilefix()
    from contextlib import ExitStack
    import concourse.bass as bass
    import concourse.tile as tile
    from concourse import mybir

    dt = mybir.dt
    bf = dt.bfloat16

    nc = bass.Bass("TRN2", target_bir_lowering=False, debug=False,
                   num_devices=8)

    # DRAM I/O (per core)
    xt_d = nc.dram_tensor("xt", [D, N], bf, kind="ExternalInput")
    x2t_d = nc.dram_tensor("x2t", [D, N], bf, kind="ExternalInput")
    wqk_d = nc.dram_tensor("wqk", [D, 2 * D], bf, kind="ExternalInput")
    wv_d = nc.dram_tensor("wv", [D, D], bf, kind="ExternalInput")
    wp_d = nc.dram_tensor("wp", [D, D], bf, kind="ExternalInput")
    bias_d = nc.dram_tensor("bias", [P, D], dt.float32, kind="ExternalInput")
    ones_d = nc.dram_tensor("ones", [P, NQT * H], bf, kind="ExternalInput")
    out_d = nc.dram_tensor("out", [2, N, D], dt.float32,
                           kind="ExternalOutput")

    with tile.TileContext(nc) as tc, ExitStack() as top:
        pp_s = top.enter_context(tc.tile_pool(name="ps_s", bufs=2,
                                              space="PSUM"))
        pp_o = top.enter_context(tc.tile_pool(name="ps_o", bufs=2,
                                              space="PSUM"))
        dram_rb = top.enter_context(tc.tile_pool(name="dram_rb", bufs=4,
                                                 space="DRAM"))
        persist = top.enter_context(tc.tile_pool(name="persist", bufs=1))

        # persistent SBUF tiles
        qT = persist.tile([P, NDT, N], bf, tag="qT")
        kT1 = persist.tile([P, NDT, N], bf, tag="kT1")
        kT2 = persist.tile([P, NDT, N], bf, tag="kT2")
        vaug1 = persist.tile([P, NQT, H * AUG], bf, tag="vaug1")
        vaug2 = persist.tile([P, NQT, H * AUG], bf, tag="vaug2")
        KJR = 4          # pt2 ring depth over kj (AV trails exp by <=2)
        pt2 = persist.tile([P, 2, KJR, N], bf, tag="pt2")
        ot0 = persist.tile([P, NDT, N], bf, tag="ot0")
        ot1 = persist.tile([P, NDT, N], bf, tag="ot1")
        wp_t = persist.tile([P, NDT, D], bf, tag="wp")
        bias_t = persist.tile([P, D], dt.float32, tag="bias")

        def load_ones(vaug_t):
            nc.sync.dma_start(
                out=vaug_t[:].rearrange("p t (h e) -> p t h e",
                                        e=AUG)[:, :, :, HD:AUG],
                in_=ones_d[:].rearrange("p (t h e) -> p t h e", h=H, e=1))

        # ================= phase A pools (kept open through br0) =========
        pool_x = top.enter_context(tc.tile_pool(name="xa", bufs=2))
        pool_w = top.enter_context(tc.tile_pool(name="wqk", bufs=1))
        pool_wv = top.enter_context(tc.tile_pool(name="wvp", bufs=1))

        # input DMAs in demand order: the serial prefix (q0, k1_0, v1)
        # needs xt + wqk blocks 0/6 + wv; everything else streams under
        # compute. Whole-tile DMAs — per-slice splits cost line efficiency
        # and the prefix accumulates over all input tiles anyway.
        xt_t = pool_x.tile([P, NDT, N], bf, tag="xt")
        xt_r = xt_d[:].rearrange("(i p) n -> p i n", p=P)
        nc.sync.dma_start(out=xt_t, in_=xt_r)
        wqk_t = pool_w.tile([P, NDT, 2 * D], bf, tag="wqk")
        wqk_r = wqk_d[:].rearrange("(i p) d -> p i d", p=P)
        for o in (0, NDT):
            nc.sync.dma_start(out=wqk_t[:, :, o * P:(o + 1) * P],
                              in_=wqk_r[:, :, o * P:(o + 1) * P])
        load_ones(vaug1)
        wv_t = pool_wv.tile([P, NDT, D], bf, tag="wv")
        nc.sync.dma_start(out=wv_t,
                          in_=wv_d[:].rearrange("(i p) d -> p i d", p=P))
        for o in range(2 * NDT):
            if o not in (0, NDT):
                nc.sync.dma_start(out=wqk_t[:, :, o * P:(o + 1) * P],
                                  in_=wqk_r[:, :, o * P:(o + 1) * P])
        x2t_t = pool_x.tile([P, NDT, N], bf, tag="xt")
        nc.sync.dma_start(out=x2t_t,
                          in_=x2t_d[:].rearrange("(i p) n -> p i n", p=P))
        load_ones(vaug2)
        nc.sync.dma_start(
            out=wp_t, in_=wp_d[:].rearrange("(g p) d -> p g d", p=P))
        nc.sync.dma_start(out=bias_t, in_=bias_d[:])

        def qkv_T_o(xt_src, colblk, o, dst_sb, evict):
            """one output tile [128, N] of q/k-transposed formation."""
            ps = pp_s.tile([P, N], dt.float32, tag="S")
            for i in range(NDT):
                wt = wqk_t[:, i, colblk * D + o * P: colblk * D + (o + 1) * P]
                for c in range(2):
                    nc.tensor.matmul(
                        ps[:, c * 512:(c + 1) * 512],
                        wt,
                        xt_src[:, i, c * 512:(c + 1) * 512],
                        start=(i == 0), stop=(i == NDT - 1))
            evict(dst_sb[:, o, :], ps[:])

        def v_t(xt_src, vaug_t, t, evict):
            """one token tile [128, D] of v formation into vaug."""
            ps = pp_s.tile([P, D], dt.float32, tag="S")
            for i in range(NDT):
                for c0, cn in ((0, 512), (512, 256)):
                    nc.tensor.matmul(
                        ps[:, c0:c0 + cn],
                        xt_src[:, i, t * P:(t + 1) * P],
                        wv_t[:, i, c0:c0 + cn],
                        start=(i == 0), stop=(i == NDT - 1))
            dstv = vaug_t[:, t, :].rearrange(
                "p (h e) -> p h e", e=AUG)[:, :, 0:HD]
            evict(dstv, ps[:].rearrange("p (h e) -> p h e", e=HD))

        # HAM warm-up: ~5us of discarded matmuls as soon as the first wqk
        # block lands, so the real prefix runs at 2.4GHz instead of 1.2.
        warm = pp_s.tile([P, 512], dt.float32, tag="S", name="warm")
        for _ in range(24):
            nc.tensor.matmul(warm[:], xt_t[:, 0, 0:P],
                             xt_t[:, 0, 0:512], start=True, stop=True,
                             skip_group_check=True)

        # phase A serial prefix: only what branch-0's very first AV needs —
        # q(o=0), k1(o=0), v1(t=0). v1(t) for t>=1 threads into g=0's kj
        # loop (AV runs one kj behind, so v1(t) lands just in time).
        # ACT evictions (ACT is free here).
        qkv_T_o(xt_t, 0, 0, qT, nc.scalar.copy)
        qkv_T_o(xt_t, 1, 0, kT1, nc.scalar.copy)
        v_t(xt_t, vaug1, 0, nc.scalar.copy)
        kj_extra = [(lambda t=t: v_t(xt_t, vaug1, t, nc.vector.tensor_copy))
                    for t in range(1, NQT)]

        # remaining QKV work rides branch-0 attention's PE slack (the g loop
        # is ACT-bound): q/k1 tile o is needed by g=o, so pairs go first in
        # order; k2/v2 must finish before branch-1 starts. DVE evictions
        # (ACT is saturated by exp there).
        thunks0 = []
        for o in range(1, NDT):
            thunks0.append(lambda o=o: qkv_T_o(xt_t, 0, o, qT,
                                               nc.vector.tensor_copy))
            thunks0.append(lambda o=o: qkv_T_o(xt_t, 1, o, kT1,
                                               nc.vector.tensor_copy))
        for o in range(NDT):
            thunks0.append(lambda o=o: qkv_T_o(x2t_t, 1, o, kT2,
                                               nc.vector.tensor_copy))
        for t in range(NQT):
            thunks0.append(lambda t=t: v_t(x2t_t, vaug2, t,
                                           nc.vector.tensor_copy))

        # ================= phase B: attention + proj ====================
        pool_osb = top.enter_context(tc.tile_pool(name="osb", bufs=4))
        pool_rc = top.enter_context(tc.tile_pool(name="rc", bufs=4))
        pool_rb = top.enter_context(tc.tile_pool(name="rb", bufs=2))
        pool_res = top.enter_context(tc.tile_pool(name="res", bufs=2))
        pool_otm = top.enter_context(tc.tile_pool(name="otm", bufs=2))

        def proj_qi(ot_t, br, qi):
            ps = pp_o.tile([P, D], dt.float32, tag="O")
            for g in range(NDT):
                for c0, cn in ((0, 512), (512, 256)):
                    nc.tensor.matmul(
                        ps[:, c0:c0 + cn],
                        ot_t[:, g, qi * P:(qi + 1) * P],
                        wp_t[:, g, c0:c0 + cn],
                        start=(g == 0), stop=(g == NDT - 1))
            res = pool_res.tile([P, D], dt.float32, tag="res")
            nc.vector.tensor_add(res[:], ps[:], bias_t[:])
            nc.sync.dma_start(out=out_d[br, qi * P:(qi + 1) * P, :],
                              in_=res[:])

        def attention(kT_t, vaug_t, ot, extra, budget, kj_extra=()):
            """head-pair ladder over g; `extra` thunks fill PE slack.
            `budget` = max extra thunks to pop per g iteration. The
            normalize tail of g is deferred one iteration so its DVE ops
            never sit in front of g+1's PSUM eviction in the FIFO (the rb
            broadcast's DMA latency is hidden by then)."""
            pending = []
            for g in range(NDT):
                po = [pp_o.tile([AUG, N], dt.float32, tag="O",
                                name=f"po{g}_{hh}") for hh in range(2)]

                def emit_av(kj):
                    for hh in range(2):
                        h = 2 * g + hh
                        for c in range(2):
                            nc.tensor.matmul(
                                po[hh][:, c * 512:(c + 1) * 512],
                                vaug_t[:, kj, h * AUG:(h + 1) * AUG],
                                pt2[:, hh, kj % KJR, c * 512:(c + 1) * 512],
                                start=(kj == 0), stop=(kj == NQT - 1),
                                skip_group_check=True)

                for kj in range(NQT):
                    for c in range(2):
                        ps = pp_s.tile([P, 2, 512], dt.float32, tag="S")
                        nc.tensor.matmul(
                            ps[:, 0, :],
                            kT_t[0:HD, g, kj * P:(kj + 1) * P],
                            qT[0:HD, g, c * 512:(c + 1) * 512],
                            start=True, stop=True)
                        nc.tensor.matmul(
                            ps[:, 1, :],
                            kT_t[HD:P, g, kj * P:(kj + 1) * P],
                            qT[HD:P, g, c * 512:(c + 1) * 512],
                            start=True, stop=True)
                        nc.scalar.activation(
                            pt2[:, :, kj % KJR, c * 512:(c + 1) * 512],
                            ps[:],
                            mybir.ActivationFunctionType.Exp, scale=SCALE)
                    if kj_extra:
                        kj_extra.pop(0)()
                    if kj >= 1:
                        emit_av(kj - 1)
                emit_av(NQT - 1)

                # evict po fast (frees PSUM); the full normalize chain is
                # deferred so it never sits ahead of the next g's evicts.
                # Last g: hh1 evict runs on the now-idle ACT so the tail
                # chain starts ~1.2us earlier.
                last = (g == NDT - 1)
                osb = [pool_osb.tile([AUG, N], dt.float32, tag="osb",
                                     name=f"osb{g}_{hh}")
                       for hh in range(2)]
                nc.vector.tensor_copy(osb[0][:], po[0][:])
                (nc.scalar.copy if last else nc.vector.tensor_copy)(
                    osb[1][:], po[1][:])

                def normalize(g=g, osb=osb):
                    rdrm = dram_rb.tile([2, N], dt.float32, tag="rd")
                    for hh in range(2):
                        nc.sync.dma_start(out=rdrm[hh, :],
                                          in_=osb[hh][HD:HD + 1, :])
                    rcol = pool_rc.tile([P, 2, 8], dt.float32, tag="rc")
                    nc.sync.dma_start(
                        out=rcol,
                        in_=rdrm[:].rearrange("a (p c) -> p a c", p=P))
                    rcol2 = pool_rc.tile([P, 2, 8], dt.float32, tag="rc2")
                    nc.vector.reciprocal(rcol2[:], rcol[:])
                    rdrm2 = dram_rb.tile([2, N], dt.float32, tag="rd2")
                    nc.sync.dma_start(
                        out=rdrm2[:].rearrange("a (p c) -> p a c", p=P),
                        in_=rcol2)
                    rb = pool_rb.tile([HD, 2, N], dt.float32, tag="rb")
                    nc.sync.dma_start(
                        out=rb[:, 0, :],
                        in_=rdrm2[0, :].partition_broadcast(HD))
                    nc.sync.dma_start(
                        out=rb[:, 1, :],
                        in_=rdrm2[1, :].partition_broadcast(HD))
                    # DVE operands must share partitions 0:HD; hh=1's
                    # result is partition-shifted into ot[HD:] by DMA.
                    nc.vector.tensor_mul(ot[0:HD, g, :], osb[0][0:HD, :],
                                         rb[:, 0, :])
                    otm = pool_otm.tile([HD, N], bf, tag="otm")
                    nc.vector.tensor_mul(otm[:], osb[1][0:HD, :],
                                         rb[:, 1, :])
                    nc.sync.dma_start(out=ot[HD:P, g, :], in_=otm[:])

                pending.append(normalize)
                if len(pending) > 1:
                    pending.pop(0)()
                for _ in range(budget):
                    if extra:
                        extra.pop(0)()
            while extra:
                extra.pop(0)()
            while pending:
                pending.pop(0)()

        attention(kT1, vaug1, ot0, thunks0, budget=5, kj_extra=kj_extra)
        # branch-1 is ACT-paced without extra PE work, so all 8 proj(br0)
        # tiles drain in `while extra` right after the last AV — before the
        # deferred-normalize flush — filling the PE-idle window of
        # branch-1's final normalize chain with ~14us of ready work.
        extra = [(lambda qi=qi: proj_qi(ot0, 0, qi)) for qi in range(NQT)]
        attention(kT2, vaug2, ot1, extra, budget=0)
        for qi in range(NQT):
            proj_qi(ot1, 1, qi)

    n = _split_multiwaits(nc)
    _built = (nc, n)
    return _built


def _host_prep(x, x2, qkv_w, proj_w, proj_b):
    """-> list of 8 per-core input maps (bf16 operands, f32 bias)."""
    import ml_dtypes
    b16 = lambda a: np.ascontiguousarray(a).astype(ml_dtypes.bfloat16)

    xt = np.transpose(np.asarray(x), (0, 2, 1))
    x2t = np.transpose(np.asarray(x2), (0, 2, 1))
    wqk = b16(np.asarray(qkv_w)[:2 * D].T)      # [768, 1536]
    wv = b16(np.asarray(qkv_w)[2 * D:].T)       # [768, 768]
    wp = b16(np.asarray(proj_w).T)              # [768, 768]
    bias = np.broadcast_to(np.asarray(proj_b, dtype=np.float32),
                           (P, D)).copy()
    ones = np.ones((P, NQT * H), dtype=ml_dtypes.bfloat16)
    maps = []
    for c in range(B):
        maps.append({
            "xt": b16(xt[c]), "x2t": b16(x2t[c]),
            "wqk": wqk, "wv": wv, "wp": wp, "bias": bias,
            "ones": ones,
        })
    return maps


def kernel(x, x2, qkv_w, proj_w, proj_b, trace=False, tmpdir=None):
    nc, _ = _build()
    from concourse.bass_utils import run_bass_kernel_spmd
    in_maps = _host_prep(x, x2, qkv_w, proj_w, proj_b)
    res = run_bass_kernel_spmd(nc, in_maps, list(range(B)), trace=trace,
                               tmpdir=tmpdir)
    kernel.last_exec_time_ns = res.exec_time_ns
    out = np.stack([res.results[c]["out"] for c in range(B)])  # [B,2,N,D]
    out1 = np.ascontiguousarray(out[:, 0])
    out2 = np.ascontiguousarray(out[:, 1])
    return (out1, out2)


kernel.last_exec_time_ns = None


# revision 43
# speedup vs baseline: 1.0182x; 1.0182x over previous
"""Two-branch attention kernel for Trainium2 (8 NeuronCores, batch-parallel).

out1 = proj(softmax(q k^T / 8) v),  out2 = proj(softmax(q k2^T / 8) v2)
with q,k,v from x and k2,v2 from x2 (q shared across branches).

Sharding: batch dim (8) -> one batch element per core. No collectives.

Design (per core, all matmul operands bf16, PSUM f32):
  QKV:  qT/kT [dim,tok] via W-stationary matmuls; v [tok,dim] via
        x-stationary matmuls into a ones-augmented buffer (AUG=65 column
        carries softmax row sums for free). k2/v2 kept in SBUF (no DRAM
        spill); their formation is interleaved into branch-1 attention so
        the PE fills the ACT-bound softmax gaps.
  Attn: per (kj,c): S^T chunk [128,2,512] in a double-buffered PSUM pair;
        the two head-half matmuls land on PE row-groups 0/64 and run
        concurrently. exp on ScalarE (scale=1/8, no max subtraction)
        writes P^T bf16; AV (V_aug stationary) pipelined one kj behind.
  Norm: po -> osb evict on DVE; row sums bounce DRAM -> [128,2,8] so the
        reciprocal uses all partitions (~0.1us, not 6.5us); broadcast back
        and multiply into ot (bf16).
  Proj: ot tiles stationary, stream W_proj^T, add bias on DVE, DMA out.
        proj(br0) interleaved into branch-2 attention.
"""
import sys
for _p in ('/opt/trn_rl_repo',):
    if _p not in sys.path:
        sys.path.insert(0, _p)

import numpy as np

MODE = 'bf16'

B, N, D, H, HD = 8, 1024, 768, 12, 64
SCALE = HD ** -0.5
NDT = D // 128       # 6 dim tiles
NQT = N // 128       # 8 token tiles
P = 128
AUG = HD + 1         # 65: head dim + ones column for row sums


# ----------------------------------------------------------------------------
# workaround: walrus rejects >2 sem waits on one instruction; TileContext's
# tail drain carries one wait per active logical proc. Split them across
# single-wait SP nops and emit a bare drain.
def _install_tilefix():
    import bass_rust
    import concourse.tile as tile

    def _drain_and_barrier_split(self, tick_clock, wait_clock):
        gc = tick_clock.global_clock
        ticks = [gc[i] for i in range(27)]
        for i, t in enumerate(ticks):
            if t > 0:
                vc = bass_rust.VectorClock(
                    [t if j == i else 0 for j in range(len(ticks))])
                nop = self.nc.sync.nop()
                wait_clock.add_sem_waits(
                    nop.ins, bass_rust.ScopedClock({None: vc}))
        self.nc.sync.drain()
        self.nc.all_engine_barrier()
        assert self.sems is not None
        popped = self.nc._tile_sem_poison_stack.pop()
        assert popped is self._sem_poison
        self.nc.clear_and_free_semaphores(list(self.sems.allocated().values()))
        self.nc.all_engine_barrier()

    tile.TileContext._drain_and_barrier = _drain_and_barrier_split


def _split_multiwaits(nc, max_waits=1):
    """walrus codegen rejects instructions carrying more than `max_waits`
    sync waits; hoist the extras onto same-engine nops placed just before."""
    import bass_rust
    import concourse.mybir as mybir
    cnt = 0
    for bb in nc.main_func.blocks:
        insts = bb.instructions
        i = 0
        while i < len(insts):
            ins = insts[i]
            si = getattr(ins, 'sync_info', None)
            if si is not None and si.on_wait and len(si.on_wait) > max_waits:
                waits = list(si.on_wait)
                extras, keep = waits[:-max_waits], waits[-max_waits:]
                for w in extras:
                    nop = mybir.InstNoOp(name=f"I-swx{cnt}", ins=[], outs=[])
                    cnt += 1
                    nop.engine = ins.engine
                    nop.sync_info = bass_rust.SyncInfo(on_wait=[w],
                                                       on_update=[])
                    insts.insert(i, nop)
                    i += 1
                ins.sync_info = bass_rust.SyncInfo(
                    on_wait=keep, on_update=list(si.on_update))
            i += 1
    return cnt


_built = None


def _build():
    """Build the SPMD bass program once. Returns (nc, n_split_waits)."""
    global _built
    if _built is not None:
        return _built
    _install_tilefix()
    from contextlib import ExitStack
    import concourse.bass as bass
    import concourse.tile as tile
    from concourse import mybir

    dt = mybir.dt
    bf = dt.bfloat16

    nc = bass.Bass("TRN2", target_bir_lowering=False, debug=False,
                   num_devices=8)

    # DRAM I/O (per core)
    xt_d = nc.dram_tensor("xt", [D, N], bf, kind="ExternalInput")
    x2t_d = nc.dram_tensor("x2t", [D, N], bf, kind="ExternalInput")
    wqk_d = nc.dram_tensor("wqk", [D, 2 * D], bf, kind="ExternalInput")
    wv_d = nc.dram_tensor("wv", [D, D], bf, kind="ExternalInput")
    wp_d = nc.dram_tensor("wp", [D, D], bf, kind="ExternalInput")
    bias_d = nc.dram_tensor("bias", [P, D], dt.float32, kind="ExternalInput")
    ones_d = nc.dram_tensor("ones", [P, NQT * H], bf, kind="ExternalInput")
    out_d = nc.dram_tensor("out", [2, N, D], dt.float32,
                           kind="ExternalOutput")

    with tile.TileContext(nc) as tc, ExitStack() as top:
        pp_s = top.enter_context(tc.tile_pool(name="ps_s", bufs=2,
                                              space="PSUM"))
        pp_o = top.enter_context(tc.tile_pool(name="ps_o", bufs=2,
                                              space="PSUM"))
        dram_rb = top.enter_context(tc.tile_pool(name="dram_rb", bufs=4,
                                                 space="DRAM"))
        persist = top.enter_context(tc.tile_pool(name="persist", bufs=1))

        # persistent SBUF tiles
        qT = persist.tile([P, NDT, N], bf, tag="qT")
        kT1 = persist.tile([P, NDT, N], bf, tag="kT1")
        kT2 = persist.tile([P, NDT, N], bf, tag="kT2")
        vaug1 = persist.tile([P, NQT, H * AUG], bf, tag="vaug1")
        vaug2 = persist.tile([P, NQT, H * AUG], bf, tag="vaug2")
        KJR = 4          # pt2 ring depth over kj (AV trails exp by <=2)
        pt2 = persist.tile([P, 2, KJR, N], bf, tag="pt2")
        ot0 = persist.tile([P, NDT, N], bf, tag="ot0")
        ot1 = persist.tile([P, NDT, N], bf, tag="ot1")
        wp_t = persist.tile([P, NDT, D], bf, tag="wp")
        bias_t = persist.tile([P, D], dt.float32, tag="bias")

        def load_ones(vaug_t):
            nc.sync.dma_start(
                out=vaug_t[:].rearrange("p t (h e) -> p t h e",
                                        e=AUG)[:, :, :, HD:AUG],
                in_=ones_d[:].rearrange("p (t h e) -> p t h e", h=H, e=1))

        # ================= phase A pools (kept open through br0) =========
        pool_x = top.enter_context(tc.tile_pool(name="xa", bufs=2))
        pool_w = top.enter_context(tc.tile_pool(name="wqk", bufs=1))
        pool_wv = top.enter_context(tc.tile_pool(name="wvp", bufs=1))

        # input DMAs in demand order: the serial prefix (q0, k1_0, v1)
        # needs xt + wqk blocks 0/6 + wv; everything else streams under
        # compute. Whole-tile DMAs — per-slice splits cost line efficiency
        # and the prefix accumulates over all input tiles anyway.
        xt_t = pool_x.tile([P, NDT, N], bf, tag="xt")
        xt_r = xt_d[:].rearrange("(i p) n -> p i n", p=P)
        nc.sync.dma_start(out=xt_t, in_=xt_r)
        wqk_t = pool_w.tile([P, NDT, 2 * D], bf, tag="wqk")
        wqk_r = wqk_d[:].rearrange("(i p) d -> p i d", p=P)
        for o in (0, NDT):
            nc.sync.dma_start(out=wqk_t[:, :, o * P:(o + 1) * P],
                              in_=wqk_r[:, :, o * P:(o + 1) * P])
        load_ones(vaug1)
        wv_t = pool_wv.tile([P, NDT, D], bf, tag="wv")
        nc.sync.dma_start(out=wv_t,
                          in_=wv_d[:].rearrange("(i p) d -> p i d", p=P))
        for o in range(2 * NDT):
            if o not in (0, NDT):
                nc.sync.dma_start(out=wqk_t[:, :, o * P:(o + 1) * P],
                                  in_=wqk_r[:, :, o * P:(o + 1) * P])
        x2t_t = pool_x.tile([P, NDT, N], bf, tag="xt")
        nc.sync.dma_start(out=x2t_t,
                          in_=x2t_d[:].rearrange("(i p) n -> p i n", p=P))
        load_ones(vaug2)
        nc.sync.dma_start(
            out=wp_t, in_=wp_d[:].rearrange("(g p) d -> p g d", p=P))
        nc.sync.dma_start(out=bias_t, in_=bias_d[:])

        def qkv_T_o(xt_src, colblk, o, dst_sb, evict):
            """one output tile [128, N] of q/k-transposed formation."""
            ps = pp_s.tile([P, N], dt.float32, tag="S")
            for i in range(NDT):
                wt = wqk_t[:, i, colblk * D + o * P: colblk * D + (o + 1) * P]
                for c in range(2):
                    nc.tensor.matmul(
                        ps[:, c * 512:(c + 1) * 512],
                        wt,
                        xt_src[:, i, c * 512:(c + 1) * 512],
                        start=(i == 0), stop=(i == NDT - 1))
            evict(dst_sb[:, o, :], ps[:])

        def v_t(xt_src, vaug_t, t, evict):
            """one token tile [128, D] of v formation into vaug."""
            ps = pp_s.tile([P, D], dt.float32, tag="S")
            for i in range(NDT):
                for c0, cn in ((0, 512), (512, 256)):
                    nc.tensor.matmul(
                        ps[:, c0:c0 + cn],
                        xt_src[:, i, t * P:(t + 1) * P],
                        wv_t[:, i, c0:c0 + cn],
                        start=(i == 0), stop=(i == NDT - 1))
            dstv = vaug_t[:, t, :].rearrange(
                "p (h e) -> p h e", e=AUG)[:, :, 0:HD]
            evict(dstv, ps[:].rearrange("p (h e) -> p h e", e=HD))

        # HAM warm-up: ~5us of discarded matmuls as soon as the first wqk
        # block lands, so the real prefix runs at 2.4GHz instead of 1.2.
        warm = pp_s.tile([P, 512], dt.float32, tag="S", name="warm")
        for _ in range(24):
            nc.tensor.matmul(warm[:], xt_t[:, 0, 0:P],
                             xt_t[:, 0, 0:512], start=True, stop=True,
                             skip_group_check=True)

        # phase A serial prefix: only what branch-0's very first AV needs —
        # q(o=0), k1(o=0), v1(t=0). v1(t) for t>=1 threads into g=0's kj
        # loop (AV runs one kj behind, so v1(t) lands just in time).
        # ACT evictions (ACT is free here).
        qkv_T_o(xt_t, 0, 0, qT, nc.scalar.copy)
        qkv_T_o(xt_t, 1, 0, kT1, nc.scalar.copy)
        v_t(xt_t, vaug1, 0, nc.scalar.copy)
        kj_extra = [(lambda t=t: v_t(xt_t, vaug1, t, nc.vector.tensor_copy))
                    for t in range(1, NQT)]

        # remaining QKV work rides branch-0 attention's PE slack (the g loop
        # is ACT-bound): q/k1 tile o is needed by g=o, so pairs go first in
        # order; k2/v2 must finish before branch-1 starts. DVE evictions
        # (ACT is saturated by exp there).
        thunks0 = []
        for o in range(1, NDT):
            thunks0.append(lambda o=o: qkv_T_o(xt_t, 0, o, qT,
                                               nc.vector.tensor_copy))
            thunks0.append(lambda o=o: qkv_T_o(xt_t, 1, o, kT1,
                                               nc.vector.tensor_copy))
        for o in range(NDT):
            thunks0.append(lambda o=o: qkv_T_o(x2t_t, 1, o, kT2,
                                               nc.vector.tensor_copy))
        for t in range(NQT):
            thunks0.append(lambda t=t: v_t(x2t_t, vaug2, t,
                                           nc.vector.tensor_copy))

        # ================= phase B: attention + proj ====================
        pool_osb = top.enter_context(tc.tile_pool(name="osb", bufs=4))
        pool_rc = top.enter_context(tc.tile_pool(name="rc", bufs=4))
        pool_rb = top.enter_context(tc.tile_pool(name="rb", bufs=2))
        pool_res = top.enter_context(tc.tile_pool(name="res", bufs=2))
        pool_otm = top.enter_context(tc.tile_pool(name="otm", bufs=2))

        def proj_qi(ot_t, br, qi, pool=None):
            # default pp_o (rotates with po during attention interleave);
            # tail projs use pp_s (tag "S" — pools ring-buffer per tag),
            # which is idle once the last exp is done, so they never wait
            # on po evictions.
            ps = ((pool or pp_o)
                  .tile([P, D], dt.float32,
                        tag="S" if pool is pp_s else "O", name=f"prj{br}{qi}"))
            for g in range(NDT):
                for c0, cn in ((0, 512), (512, 256)):
                    nc.tensor.matmul(
                        ps[:, c0:c0 + cn],
                        ot_t[:, g, qi * P:(qi + 1) * P],
                        wp_t[:, g, c0:c0 + cn],
                        start=(g == 0), stop=(g == NDT - 1))
            res = pool_res.tile([P, D], dt.float32, tag="res")
            nc.vector.tensor_add(res[:], ps[:], bias_t[:])
            nc.sync.dma_start(out=out_d[br, qi * P:(qi + 1) * P, :],
                              in_=res[:])

        def attention(kT_t, vaug_t, ot, extra, budget, kj_extra=()):
            """head-pair ladder over g; `extra` thunks fill PE slack.
            `budget` = max extra thunks to pop per g iteration. The
            normalize tail of g is deferred one iteration so its DVE ops
            never sit in front of g+1's PSUM eviction in the FIFO (the rb
            broadcast's DMA latency is hidden by then)."""
            pending = []
            for g in range(NDT):
                po = [pp_o.tile([AUG, N], dt.float32, tag="O",
                                name=f"po{g}_{hh}") for hh in range(2)]

                def emit_av(kj):
                    for hh in range(2):
                        h = 2 * g + hh
                        for c in range(2):
                            nc.tensor.matmul(
                                po[hh][:, c * 512:(c + 1) * 512],
                                vaug_t[:, kj, h * AUG:(h + 1) * AUG],
                                pt2[:, hh, kj % KJR, c * 512:(c + 1) * 512],
                                start=(kj == 0), stop=(kj == NQT - 1),
                                skip_group_check=True)

                for kj in range(NQT):
                    for c in range(2):
                        ps = pp_s.tile([P, 2, 512], dt.float32, tag="S")
                        nc.tensor.matmul(
                            ps[:, 0, :],
                            kT_t[0:HD, g, kj * P:(kj + 1) * P],
                            qT[0:HD, g, c * 512:(c + 1) * 512],
                            start=True, stop=True)
                        nc.tensor.matmul(
                            ps[:, 1, :],
                            kT_t[HD:P, g, kj * P:(kj + 1) * P],
                            qT[HD:P, g, c * 512:(c + 1) * 512],
                            start=True, stop=True)
                        nc.scalar.activation(
                            pt2[:, :, kj % KJR, c * 512:(c + 1) * 512],
                            ps[:],
                            mybir.ActivationFunctionType.Exp, scale=SCALE)
                    if kj_extra:
                        kj_extra.pop(0)()
                    if kj >= 1:
                        emit_av(kj - 1)
                emit_av(NQT - 1)

                # evict po fast (frees PSUM); the full normalize chain is
                # deferred so it never sits ahead of the next g's evicts.
                # Last g: hh1 evict runs on the now-idle ACT so the tail
                # chain starts ~1.2us earlier.
                last = (g == NDT - 1)
                osb = [pool_osb.tile([AUG, N], dt.float32, tag="osb",
                                     name=f"osb{g}_{hh}")
                       for hh in range(2)]
                nc.vector.tensor_copy(osb[0][:], po[0][:])
                (nc.scalar.copy if last else nc.vector.tensor_copy)(
                    osb[1][:], po[1][:])

                def normalize(g=g, osb=osb):
                    rdrm = dram_rb.tile([2, N], dt.float32, tag="rd")
                    for hh in range(2):
                        nc.sync.dma_start(out=rdrm[hh, :],
                                          in_=osb[hh][HD:HD + 1, :])
                    rcol = pool_rc.tile([P, 2, 8], dt.float32, tag="rc")
                    nc.sync.dma_start(
                        out=rcol,
                        in_=rdrm[:].rearrange("a (p c) -> p a c", p=P))
                    rcol2 = pool_rc.tile([P, 2, 8], dt.float32, tag="rc2")
                    nc.vector.reciprocal(rcol2[:], rcol[:])
                    rdrm2 = dram_rb.tile([2, N], dt.float32, tag="rd2")
                    nc.sync.dma_start(
                        out=rdrm2[:].rearrange("a (p c) -> p a c", p=P),
                        in_=rcol2)
                    rb = pool_rb.tile([HD, 2, N], dt.float32, tag="rb")
                    nc.sync.dma_start(
                        out=rb[:, 0, :],
                        in_=rdrm2[0, :].partition_broadcast(HD))
                    nc.sync.dma_start(
                        out=rb[:, 1, :],
                        in_=rdrm2[1, :].partition_broadcast(HD))
                    # DVE operands must share partitions 0:HD; hh=1's
                    # result is partition-shifted into ot[HD:] by DMA.
                    nc.vector.tensor_mul(ot[0:HD, g, :], osb[0][0:HD, :],
                                         rb[:, 0, :])
                    otm = pool_otm.tile([HD, N], bf, tag="otm")
                    nc.vector.tensor_mul(otm[:], osb[1][0:HD, :],
                                         rb[:, 1, :])
                    nc.sync.dma_start(out=ot[HD:P, g, :], in_=otm[:])

                pending.append(normalize)
                if len(pending) > 1:
                    pending.pop(0)()
                for _ in range(budget):
                    if extra:
                        extra.pop(0)()
            while extra:
                extra.pop(0)()
            while pending:
                pending.pop(0)()

        attention(kT1, vaug1, ot0, thunks0, budget=5, kj_extra=kj_extra)
        # branch-1 is ACT-paced without extra PE work, so all 8 proj(br0)
        # tiles drain in `while extra` right after the last AV — before the
        # deferred-normalize flush — filling the PE-idle window of
        # branch-1's final normalize chain with ~14us of ready work.
        extra = [(lambda qi=qi: proj_qi(ot0, 0, qi, pp_s))
                 for qi in range(NQT)]
        attention(kT2, vaug2, ot1, extra, budget=0)
        for qi in range(NQT):
            proj_qi(ot1, 1, qi, pp_s)

    n = _split_multiwaits(nc)
    _built = (nc, n)
    return _built


def _host_prep(x, x2, qkv_w, proj_w, proj_b):
    """-> list of 8 per-core input maps (bf16 operands, f32 bias)."""
    import ml_dtypes
    b16 = lambda a: np.ascontiguousarray(a).astype(ml_dtypes.bfloat16)

    xt = np.transpose(np.asarray(x), (0, 2, 1))
    x2t = np.transpose(np.asarray(x2), (0, 2, 1))
    wqk = b16(np.asarray(qkv_w)[:2 * D].T)      # [768, 1536]
    wv = b16(np.asarray(qkv_w)[2 * D:].T)       # [768, 768]
    wp = b16(np.asarray(proj_w).T)              # [768, 768]
    bias = np.broadcast_to(np.asarray(proj_b, dtype=np.float32),
                           (P, D)).copy()
    ones = np.ones((P, NQT * H), dtype=ml_dtypes.bfloat16)
    maps = []
    for c in range(B):
        maps.append({
            "xt": b16(xt[c]), "x2t": b16(x2t[c]),
            "wqk": wqk, "wv": wv, "wp": wp, "bias": bias,
            "ones": ones,
        })
    return maps


def kernel(x, x2, qkv_w, proj_w, proj_b, trace=False, tmpdir=None):
    nc, _ = _build()
    from concourse.bass_utils import run_bass_kernel_spmd
    in_maps = _host_prep(x, x2, qkv_w, proj_w, proj_b)
    res = run_bass_kernel_spmd(nc, in_maps, list(range(B)), trace=trace,
                               tmpdir=tmpdir)
    kernel.last_exec_time_ns = res.exec_time_ns
    out = np.stack([res.results[c]["out"] for c in range(B)])  # [B,2,N,D]
    out1 = np.ascontiguousarray(out[:, 0])
    out2 = np.ascontiguousarray(out[:, 1])
    return (out1, out2)


kernel.last_exec_time_ns = None
